# revision 4
# baseline (speedup 1.0000x reference)
"""GATv2 (2-layer, PyG-style) on 8 Trainium2 NeuronCores via Bass — v2.

Strategy (edge-parallel over dst-sorted edges, node-range sharded):
  - Core c owns dst nodes [c*1250, (c+1)*1250) and their incoming edges
    (~20k each), grouped in windows of 125 nodes; each window's edges are
    padded to whole 128-edge tiles (pad edges gather row 0 and carry score
    bias -60 so exp() == 0 in fp16).
  - All node tables and the edge pipeline run in fp16 with an internal
    c-major channel permutation (col = c*8+h) so the per-head score
    reduction becomes a log2 fold tree of dense 2x-mode DVE adds.
  - Row gathers use windowed SWDGE dma_gather (int16 indices, up to 8
    tiles per call) instead of per-tile indirect DMA: ~1.3us per 1024 rows
    of descriptor-gen instead of 1.8us per 128 rows.
  - Scatter-add onto dst nodes via one-hot matmuls into PSUM (one-hot
    matrices precomputed on the host and DMA-streamed).
  - LeakyReLU runs as Prelu on the scalar engine: parametric_relu shares
    an activation-table set with exp/ln/relu/copy, so the scalar engine
    never reloads tables in steady state.
  - ELU via h' = relu(x)+exp(min(x,0)) = elu(x)+1; the -1 folds into
    layer-2's node transforms (xl2 = h'@W2_l - colsum(W2_l)).
  - Layer-2 node transforms run inline per window: h' is transposed by
    DMA-transpose and hit with one [128,125,64] matmul per 128-chunk
    producing [xl2|xr2] together.
  - Only collective: AllGather of the combined [1250,128] fp16 xl2/xr2
    rows. Layer 2 repeats the edge pass with H=1, C=32 (both operands
    gathered; U2 and D2 fused into one [128,125,33] matmul), then
    log_softmax with a fused exp+rowsum on the scalar engine.
"""

import sys
from contextlib import ExitStack

sys.path.insert(0, "/opt/trn_rl_repo")

import numpy as np

import concourse.bass as bass
import concourse.tile as tile
from concourse import mybir
from concourse.bass_utils import run_bass_kernel_spmd
from concourse.library_config import mlp as _mlp_lib

import ml_dtypes
F16NP = ml_dtypes.bfloat16
F32 = mybir.dt.float32
F16 = mybir.dt.bfloat16
I16 = mybir.dt.int16
AF = mybir.ActivationFunctionType
OP = mybir.AluOpType

NEG_SLOPE = 0.2
PAD_BIAS = -60.0
GT = 4  # tiles per gather group


class Cfg:
    def __init__(self, n, in_ch, hid, heads, out_ch, ncores, win, nwin):
        self.n = n
        self.in_ch = in_ch
        self.hid = hid
        self.heads = heads
        self.out = out_ch            # 32
        self.outp = 128              # padded layer-2 row (256B)
        self.ncores = ncores
        self.win = win               # nodes per window (<=128)
        self.nwin = nwin
        self.npc = win * nwin
        self.kc = in_ch // 128
        self.hc = heads * hid        # 1024
        self.hcc = self.hc // 128    # 8
        assert self.npc * ncores == n


def _split_sync_waits(nc, cap=2):
    """Walrus in this container rejects instructions with more than a couple
    of semaphore waits; hoist the excess onto preceding same-engine NoOps."""
    import bass_rust

    n_new = 0
    for f in nc.m.functions:
        for b in f.blocks:
            out = []
            for inst in b.instructions:
                si = getattr(inst, "sync_info", None)
                waits = list(si.on_wait) if si is not None and si.on_wait else []
                if len(waits) > cap:
                    keep, extra = waits[-cap:], waits[:-cap]
                    while extra:
                        chunk, extra = extra[:cap], extra[cap:]
                        n_new += 1
                        nop = bass_rust.InstNoOp(
                            name=f"I-wsplit-{n_new}", engine=inst.engine, ins=[], outs=[]
                        )
                        nop.sync_info = mybir.SyncInfo(on_wait=chunk, on_update=[])
                        try:
                            nop.bass_nofuse = True
                        except Exception:
                            pass
                        try:
                            nc.register_instruction(nop, overwrite=True)
                        except Exception:
                            pass
                        out.append(nop)
                    si.on_wait = keep
                out.append(inst)
            b.instructions = out
    return n_new


def build_program(cfg, Tw, add_b1, add_b2):
    nc = bass.Bass(num_devices=cfg.ncores, num_swdge_queues=4)
    T = int(sum(Tw))
    hc, win, heads, hid = cfg.hc, cfg.win, cfg.heads, cfg.hid
    out_c, outp = cfg.out, cfg.outp
    NH = (cfg.n + 127) // 128

    P = lambda name, shape, dt: nc.declare_dram_parameter(name, shape, dt, isOutput=False)
    xt = P("xt", [128, cfg.kc * cfg.n], F16)          # x^T chunk-major
    xlt = P("xlt", [128, cfg.kc * cfg.npc], F16)      # local x^T slice
    w1l = P("w1l", [128, cfg.kc * hc], F16)           # c-major cols
    w1r = P("w1r", [128, cfg.kc * hc], F16)
    w2lr = P("w2lr", [128, cfg.hcc * 64], F16)        # [W2_l | W2_r] per chunk
    att1r = P("att1r", [128, hc], F16)                # c-major, replicated
    att2r = P("att2r", [128, out_c], F16)
    clrr = P("clrr", [128, 64], F32)                  # [colsum W2_l | colsum W2_r]
    biass = P("biass", [128, T], F32)                 # pad-mask score bias
    ohs = P("ohs", [128, T * win], F16)               # one-hot dst-in-window
    ohsT = P("ohsT", [128, T * 128], F16)             # transposed one-hots
    idx_src = P("idx_src", [128, T * 8], I16)         # packed gather indices
    idx_dst = P("idx_dst", [128, T * 8], I16)
    b1r = P("b1r", [128, hc], F16) if add_b1 else None
    b2r = P("b2r", [128, out_c], F32) if add_b2 else None
    out_ext = nc.declare_dram_parameter("out", [cfg.npc, out_c], F32, isOutput=True)

    xlr2_loc = nc.dram_tensor("xlr2_loc", [cfg.npc, outp], F16)
    xlr2_full = nc.dram_tensor("xlr2_full", [cfg.n, outp], F16, addr_space="Shared")

    # window -> list of (tile0, ktiles) gather groups
    def groups_of(w):
        nt = int(Tw[w])
        return [(g0, min(GT, nt - g0)) for g0 in range(0, nt, GT)]

    # one gpsimd register per distinct num_idxs value (to_reg would
    # otherwise allocate a fresh register per dma_gather call)
    def reg_cache(stack):
        cache = {}

        def get(v):
            if v not in cache:
                r = stack.enter_context(
                    nc.gpsimd.register(f"nidx{v}_{nc.next_id()}")
                )
                nc.gpsimd.reg_mov(r, v)
                cache[v] = r
            return cache[v]

        return get

    # ================= context A =================
    with tile.TileContext(nc) as tc:
        with (
            tc.tile_pool(name="dram", bufs=1, space="DRAM") as dramp,
            tc.tile_pool(name="consts", bufs=1) as consts,
        ):
            nc.gpsimd.load_library(_mlp_lib)
            xl1_tbl = dramp.tile([cfg.n, hc], F16)
            xr1_tbl = dramp.tile([cfg.npc, hc], F16)

            bias_sb = consts.tile([128, T], F32)
            nc.sync.dma_start(out=bias_sb[:], in_=biass[:])
            oh_sb = consts.tile([128, T * win], F16)
            nc.sync.dma_start(out=oh_sb[:], in_=ohs[:])
            isrc_sb = consts.tile([128, T * 8], I16)
            nc.sync.dma_start(out=isrc_sb[:], in_=idx_src[:])
            idst_sb = consts.tile([128, T * 8], I16)
            nc.sync.dma_start(out=idst_sb[:], in_=idx_dst[:])
            att1_sb = consts.tile([128, hc], F16)
            nc.sync.dma_start(out=att1_sb[:], in_=att1r[:])
            w2lr_sb = consts.tile([128, cfg.hcc * 64], F16)
            nc.sync.dma_start(out=w2lr_sb[:], in_=w2lr[:])
            clrr_sb = consts.tile([128, 64], F32)
            nc.sync.dma_start(out=clrr_sb[:], in_=clrr[:])
            if add_b1:
                b1r_sb = consts.tile([128, hc], F16)
                nc.sync.dma_start(out=b1r_sb[:], in_=b1r[:])

            # ---- phase 1: node transforms -> xl1 (full) and xr1 (local) ----
            with (
                tc.tile_pool(name="xtp", bufs=1) as xtp,
                tc.tile_pool(name="stg", bufs=4) as stg,
                tc.tile_pool(name="psA", bufs=2, space="PSUM") as psA,
            ):
                xt_sb = xtp.tile([128, cfg.kc * cfg.n], F16)
                nc.sync.dma_start(out=xt_sb[:], in_=xt[:])
                xlt_sb = xtp.tile([128, cfg.kc * cfg.npc], F16)
                nc.sync.dma_start(out=xlt_sb[:], in_=xlt[:])
                w1l_sb = xtp.tile([128, cfg.kc * hc], F16)
                nc.sync.dma_start(out=w1l_sb[:], in_=w1l[:])
                w1r_sb = xtp.tile([128, cfg.kc * hc], F16)
                nc.sync.dma_start(out=w1r_sb[:], in_=w1r[:])

                def node_mm(ps, lhs_sb, lhs_stride, n0, M, w_sb):
                    for k in range(cfg.kc):
                        lhsT = lhs_sb[:, k * lhs_stride + n0 : k * lhs_stride + n0 + M]
                        for h2 in range(hc // 512):
                            nc.tensor.matmul(
                                out=ps[0:M, h2 * 512 : (h2 + 1) * 512],
                                lhsT=lhsT,
                                rhs=w_sb[:, k * hc + h2 * 512 : k * hc + (h2 + 1) * 512],
                                start=(k == 0),
                                stop=(k == cfg.kc - 1),
                            )

                for i in range(NH):
                    n0 = i * 128
                    M = min(128, cfg.n - n0)
                    ps = psA.tile([128, hc], F32)
                    node_mm(ps, xt_sb, cfg.n, n0, M, w1l_sb)
                    stage = stg.tile([128, hc], F16, tag="stage")
                    if i % 2 == 0:
                        nc.scalar.copy(out=stage[0:M, :], in_=ps[0:M, :])
                    else:
                        nc.vector.tensor_copy(out=stage[0:M, :], in_=ps[0:M, :])
                    nc.sync.dma_start(out=xl1_tbl[n0 : n0 + M, :], in_=stage[0:M, :])

                for w in range(cfg.nwin):
                    ps = psA.tile([128, hc], F32)
                    node_mm(ps, xlt_sb, cfg.npc, w * win, win, w1r_sb)
                    stage = stg.tile([128, hc], F16, tag="stage")
                    if w % 2 == 0:
                        nc.scalar.copy(out=stage[0:win, :], in_=ps[0:win, :])
                    else:
                        nc.vector.tensor_copy(out=stage[0:win, :], in_=ps[0:win, :])
                    nc.sync.dma_start(
                        out=xr1_tbl[w * win : (w + 1) * win, :], in_=stage[0:win, :]
                    )

            # ---- phase 2+3: layer-1 edges, ELU, layer-2 transforms ----
            with (
                ExitStack() as rstack,
                tc.tile_pool(name="hwin", bufs=1) as hp,
                tc.tile_pool(name="xlg", bufs=4) as xlp,
                tc.tile_pool(name="xrg", bufs=4) as xrp,
                tc.tile_pool(name="sc", bufs=5) as sp,
                tc.tile_pool(name="ep", bufs=2) as ep,
                tc.tile_pool(name="hT", bufs=2) as htp,
                tc.tile_pool(name="psU", bufs=2, space="PSUM") as psU,
                tc.tile_pool(name="psD", bufs=2, space="PSUM") as psD,
                tc.tile_pool(name="ps64", bufs=2, space="PSUM") as ps64p,
            ):
                nreg = reg_cache(rstack)

                def l1_tail(t, ti, xl_t, lr, U1, D1, ntile):
                    # score + weighted message for a tile whose prelu is done
                    mm = sp.tile([128, hc], F16, tag="mm")
                    nc.vector.tensor_mul(out=mm[:], in0=lr[:], in1=att1_sb[:])
                    # c-major fold tree: sum over c (outer), keep h (inner 8)
                    fa = sp.tile([128, 512], F16, tag="fa")
                    nc.vector.tensor_add(
                        out=fa[:], in0=mm[:, 0:512], in1=mm[:, 512:1024]
                    )
                    nc.vector.tensor_add(
                        out=fa[:, 0:256], in0=fa[:, 0:256], in1=fa[:, 256:512]
                    )
                    fb = sp.tile([128, 128], F32, tag="fb")
                    nc.vector.tensor_add(
                        out=fb[:], in0=fa[:, 0:128], in1=fa[:, 128:256]
                    )
                    nc.vector.tensor_add(
                        out=fb[:, 0:64], in0=fb[:, 0:64], in1=fb[:, 64:128]
                    )
                    nc.vector.tensor_add(
                        out=fb[:, 0:32], in0=fb[:, 0:32], in1=fb[:, 32:64]
                    )
                    nc.vector.tensor_add(
                        out=fb[:, 0:16], in0=fb[:, 0:16], in1=fb[:, 16:32]
                    )
                    e = sp.tile([128, heads], F32, tag="e")
                    nc.vector.tensor_add(out=e[:], in0=fb[:, 0:8], in1=fb[:, 8:16])
                    w_bf = sp.tile([128, heads], F16, tag="wbf")
                    nc.scalar.activation(
                        out=w_bf[:], in_=e[:], func=AF.Exp,
                        bias=bias_sb[:, ti : ti + 1], scale=1.0,
                    )
                    rhsw = sp.tile([128, hc], F16, tag="rhsw")
                    nc.vector.tensor_mul(
                        out=rhsw[:].rearrange("p (c h) -> p c h", h=heads),
                        in0=xl_t.rearrange("p (c h) -> p c h", h=heads),
                        in1=w_bf[:, None, :].to_broadcast([128, hid, heads]),
                    )
                    oh = oh_sb[:, ti * win : (ti + 1) * win]
                    st = t == 0
                    sp_ = t == ntile - 1
                    nc.tensor.matmul(
                        out=U1[0:win, 0:512], lhsT=oh, rhs=rhsw[:, 0:512],
                        start=st, stop=sp_,
                    )
                    nc.tensor.matmul(
                        out=U1[0:win, 512:1024], lhsT=oh, rhs=rhsw[:, 512:1024],
                        start=st, stop=sp_,
                    )
                    nc.tensor.matmul(
                        out=D1[0:win, 0:heads], lhsT=oh, rhs=w_bf[:],
                        start=st, stop=sp_,
                    )

                toff = 0
                gq = 0
                pend = None
                for w in range(cfg.nwin):
                    U1 = psU.tile([128, hc], F32)
                    D1 = psD.tile([128, heads], F32)
                    ntile = int(Tw[w])
                    for g0, kg in groups_of(w):
                        xlg = xlp.tile([128, GT, hc], F16, tag="xlg")
                        nc.gpsimd.dma_gather(
                            xlg[:, 0:kg, :], xl1_tbl[:],
                            isrc_sb[:, (toff + g0) * 8 : (toff + g0 + kg) * 8],
                            kg * 128, nreg(kg * 128), hc, queue_num=gq % 4,
                        )
                        xrg = xrp.tile([128, GT, hc], F16, tag="xrg")
                        nc.gpsimd.dma_gather(
                            xrg[:, 0:kg, :], xr1_tbl[:],
                            idst_sb[:, (toff + g0) * 8 : (toff + g0 + kg) * 8],
                            kg * 128, nreg(kg * 128), hc, queue_num=(gq + 1) % 4,
                        )
                        gq += 2
                        for j in range(kg):
                            t = g0 + j
                            ti = toff + t
                            xl_t = xlg[:, j, :]
                            xr_t = xrg[:, j, :]
                            m = sp.tile([128, hc], F16, tag="m")
                            nc.vector.tensor_add(out=m[:], in0=xl_t, in1=xr_t)
                            lr = sp.tile([128, hc], F16, tag="lr")
                            nc.scalar.activation(
                                out=lr[:], in_=m[:], func=AF.Prelu, alpha=NEG_SLOPE
                            )
                            if pend is not None:
                                l1_tail(*pend)
                            pend = (t, ti, xl_t, lr, U1, D1, ntile)
                    toff += ntile

                    # flush the pipelined last tile before reading U1/D1
                    if pend is not None:
                        l1_tail(*pend)
                        pend = None
                    # window epilogue: h' = relu(U/D [+b1]) + exp(min(U/D, 0))
                    deps = ep.tile([128, heads], F32, tag="deps")
                    nc.vector.tensor_scalar_add(
                        out=deps[0:win, :], in0=D1[0:win, :], scalar1=1e-16
                    )
                    rd = ep.tile([128, heads], F32, tag="rd")
                    nc.vector.reciprocal(out=rd[0:win, :], in_=deps[0:win, :])
                    hdiv = ep.tile([128, hc], F16, tag="hdiv")
                    nc.vector.tensor_mul(
                        out=hdiv[0:win, :].rearrange("p (c h) -> p c h", h=heads),
                        in0=U1[0:win, :].rearrange("p (c h) -> p c h", h=heads),
                        in1=rd[0:win, None, :].to_broadcast([win, hid, heads]),
                    )
                    if add_b1:
                        nc.vector.tensor_add(
                            out=hdiv[0:win, :], in0=hdiv[0:win, :], in1=b1r_sb[0:win, :]
                        )
                    ra = ep.tile([128, hc], F16, tag="ra")
                    nc.scalar.activation(out=ra[0:win, :], in_=hdiv[0:win, :], func=AF.Relu)
                    # exp(min(x,0)) = min(exp(x),1) = 1 - relu(1 - exp(x)),
                    # all on the scalar engine (bf16 holds exp(40) fine)
                    mn = ep.tile([128, hc], F16, tag="mn")
                    nc.scalar.activation(out=mn[0:win, :], in_=hdiv[0:win, :], func=AF.Exp)
                    exm = ep.tile([128, hc], F16, tag="exm")
                    nc.scalar.activation(
                        out=exm[0:win, :], in_=mn[0:win, :], func=AF.Relu,
                        scale=-1.0, bias=1.0,
                    )
                    hw = hp.tile([128, hc], F16, tag=f"h{w}")
                    # h' = ra + (1 - exm)
                    nc.vector.scalar_tensor_tensor(
                        out=hw[0:win, :], in0=exm[0:win, :], scalar=-1.0,
                        in1=ra[0:win, :], op0=OP.mult, op1=OP.add,
                    )
                    nc.vector.tensor_scalar_add(
                        out=hw[0:win, :], in0=hw[0:win, :], scalar1=1.0
                    )

                    # inline layer-2 node transforms for this window
                    hT = htp.tile([128, cfg.hcc, 128], F16, tag="hT")
                    for k in range(cfg.hcc):
                        nc.sync.dma_start_transpose(
                            out=hT[:, k, :], in_=hw[:, k * 128 : (k + 1) * 128]
                        )
                    ps64 = ps64p.tile([128, 64], F32)
                    for k in range(cfg.hcc):
                        nc.tensor.matmul(
                            out=ps64[0:win, :],
                            lhsT=hT[:, k, 0:win],
                            rhs=w2lr_sb[:, k * 64 : (k + 1) * 64],
                            start=(k == 0),
                            stop=(k == cfg.hcc - 1),
                        )
                    s2 = ep.tile([128, 64], F16, tag="s2")
                    nc.vector.tensor_sub(
                        out=s2[0:win, :], in0=ps64[0:win, :], in1=clrr_sb[0:win, :]
                    )
                    nc.scalar.dma_start(
                        out=xlr2_loc[w * win : (w + 1) * win, 0:64], in_=s2[0:win, :]
                    )

    # ================= collective =================
    cc_sem = nc.alloc_semaphore("cc_sem")
    nc.gpsimd.collective_compute(
        "AllGather",
        OP.bypass,
        replica_groups=[list(range(cfg.ncores))],
        ins=[xlr2_loc[:]],
        outs=[xlr2_full[:]],
    ).then_inc(cc_sem)
    nc.gpsimd.wait_ge(cc_sem, 1)
    nc.all_engine_barrier()
    nc.clear_and_free_semaphores([cc_sem])
    nc.all_engine_barrier()

    # ================= context B: layer-2 edges + log_softmax =================
    with tile.TileContext(nc) as tc:
        with (
            ExitStack() as rstack2,
            tc.tile_pool(name="c2", bufs=1) as c2,
            tc.tile_pool(name="xg2", bufs=4) as xg2p,
            tc.tile_pool(name="rg2", bufs=4) as rg2p,
            tc.tile_pool(name="sc2", bufs=4) as sp2,
            tc.tile_pool(name="psB", bufs=4, space="PSUM") as psB,
        ):
            bias2_sb = c2.tile([128, T], F32)
            nc.sync.dma_start(out=bias2_sb[:], in_=biass[:])
            oh2_sb = c2.tile([128, T * win], F16)
            nc.sync.dma_start(out=oh2_sb[:], in_=ohs[:])
            isrc2_sb = c2.tile([128, T * 8], I16)
            nc.sync.dma_start(out=isrc2_sb[:], in_=idx_src[:])
            idst2_sb = c2.tile([128, T * 8], I16)
            nc.sync.dma_start(out=idst2_sb[:], in_=idx_dst[:])
            att2_sb = c2.tile([128, out_c], F16)
            nc.sync.dma_start(out=att2_sb[:], in_=att2r[:])
            if add_b2:
                b2r_sb = c2.tile([128, out_c], F32)
                nc.sync.dma_start(out=b2r_sb[:], in_=b2r[:])

            nreg2 = reg_cache(rstack2)

            def l2_tail(t, ti, g1, e2, UD2, ntile):
                rhs2 = sp2.tile([128, 33], F16, tag="rhs2")
                w2f = sp2.tile([128, 1], F32, tag="w2f")
                nc.scalar.activation(
                    out=w2f[:], in_=e2[:], func=AF.Exp,
                    bias=bias2_sb[:, ti : ti + 1], scale=1.0,
                )
                nc.vector.tensor_copy(out=rhs2[:, 32:33], in_=w2f[:])
                nc.vector.tensor_scalar_mul(
                    out=rhs2[:, 0:32], in0=g1, scalar1=w2f[:]
                )
                nc.tensor.matmul(
                    out=UD2[0:win, :],
                    lhsT=oh2_sb[:, ti * win : (ti + 1) * win],
                    rhs=rhs2[:],
                    start=(t == 0), stop=(t == ntile - 1),
                )

            toff = 0
            gq = 0
            pend2 = None
            for w in range(cfg.nwin):
                UD2 = psB.tile([128, 33], F32)
                ntile = int(Tw[w])
                for g0, kg in groups_of(w):
                    xg = xg2p.tile([128, GT, outp], F16, tag="xg")
                    nc.gpsimd.dma_gather(
                        xg[:, 0:kg, :], xlr2_full[:],
                        isrc2_sb[:, (toff + g0) * 8 : (toff + g0 + kg) * 8],
                        kg * 128, nreg2(kg * 128), outp, queue_num=gq % 4,
                    )
                    rg = rg2p.tile([128, GT, outp], F16, tag="rg")
                    nc.gpsimd.dma_gather(
                        rg[:, 0:kg, :], xlr2_loc[:],
                        idst2_sb[:, (toff + g0) * 8 : (toff + g0 + kg) * 8],
                        kg * 128, nreg2(kg * 128), outp, queue_num=(gq + 1) % 4,
                    )
                    gq += 2
                    for j in range(kg):
                        t = g0 + j
                        ti = toff + t
                        g1 = xg[:, j, 0:out_c]
                        m2 = sp2.tile([128, out_c], F16, tag="m2")
                        nc.vector.tensor_add(
                            out=m2[:], in0=g1, in1=rg[:, j, 32:64]
                        )
                        lr2 = sp2.tile([128, out_c], F16, tag="lr2")
                        e2 = sp2.tile([128, 1], F32, tag="e2")
                        nc.vector.scalar_tensor_tensor(
                            out=lr2[:], in0=m2[:], scalar=NEG_SLOPE, in1=m2[:],
                            op0=OP.mult, op1=OP.max,
                        )
                        lm = sp2.tile([128, out_c], F16, tag="lm")
                        nc.vector.scalar_tensor_tensor(
                            out=lm[:], in0=lr2[:], scalar=1.0, in1=att2_sb[:],
                            op0=OP.mult, op1=OP.mult, accum_out=e2[:],
                        )
                        if pend2 is not None:
                            l2_tail(*pend2)
                        pend2 = (t, ti, g1, e2, UD2, ntile)
                toff += ntile

                # flush the pipelined last tile before reading UD2
                if pend2 is not None:
                    l2_tail(*pend2)
                    pend2 = None
                # epilogue: z = U2/D2 (+b2); out = z - ln(sum(exp(z)))
                d2e = sp2.tile([128, 1], F32, tag="d2e")
                nc.vector.tensor_scalar_add(
                    out=d2e[0:win, :], in0=UD2[0:win, 32:33], scalar1=1e-16
                )
                rd2 = sp2.tile([128, 1], F32, tag="rd2")
                nc.vector.reciprocal(out=rd2[0:win, :], in_=d2e[0:win, :])
                z = sp2.tile([128, out_c], F32, tag="z")
                nc.vector.tensor_scalar_mul(
                    out=z[0:win, :], in0=UD2[0:win, 0:32], scalar1=rd2[0:win, :]
                )
                if add_b2:
                    nc.vector.tensor_add(
                        out=z[0:win, :], in0=z[0:win, :], in1=b2r_sb[0:win, :]
                    )
                ez = sp2.tile([128, out_c], F16, tag="ez")
                sz = sp2.tile([128, 1], F32, tag="sz")
                nc.scalar.activation(
                    out=ez[0:win, :], in_=z[0:win, :], func=AF.Exp, accum_out=sz[0:win, :]
                )
                lz = sp2.tile([128, 1], F32, tag="lz")
                nc.scalar.activation(out=lz[0:win, :], in_=sz[0:win, :], func=AF.Ln)
                zo = sp2.tile([128, out_c], F32, tag="zo")
                nc.vector.tensor_scalar_sub(
                    out=zo[0:win, :], in0=z[0:win, :], scalar1=lz[0:win, :]
                )
                nc.sync.dma_start(
                    out=out_ext[w * win : (w + 1) * win, :], in_=zo[0:win, :]
                )
    mybir.codegen_inst_isa_subclasses(nc)
    _split_sync_waits(nc, cap=1)
    return nc


def host_prep(cfg, x, edge_index, W1_l, W1_r, att1, b1, W2_l, W2_r, att2, b2):
    src = np.asarray(edge_index[0], dtype=np.int64)
    dst = np.asarray(edge_index[1], dtype=np.int64)
    order = np.argsort(dst, kind="stable")
    src_s, dst_s = src[order], dst[order]
    deg = np.bincount(dst, minlength=cfg.n)
    cnt = deg.reshape(cfg.ncores, cfg.nwin, cfg.win).sum(axis=2)
    Tw = np.maximum(np.ceil(cnt / 128).astype(int).max(axis=0), 1)
    T = int(Tw.sum())

    starts = np.zeros(cfg.n + 1, dtype=np.int64)
    np.cumsum(deg, out=starts[1:])

    # c-major permutation: new col c*8+h <- old col h*128+c
    hh, cc = np.meshgrid(np.arange(cfg.heads), np.arange(cfg.hid), indexing="ij")
    perm = np.empty(cfg.hc, np.int64)
    perm[cc.ravel() * cfg.heads + hh.ravel()] = (hh * cfg.hid + cc).ravel()

    biass_l, ohs_l, ohsT_l, isrc_l, idst_l = [], [], [], [], []
    for c in range(cfg.ncores):
        biasc = np.full((128, T), PAD_BIAS, np.float32)
        ohc = np.zeros((128, T, cfg.win), F16NP)
        ohTc = np.zeros((128, T * 128), F16NP)
        isc = np.zeros((128, T * 8), np.int16)
        idc = np.zeros((128, T * 8), np.int16)
        toff = 0
        for w in range(cfg.nwin):
            g0 = c * cfg.npc + w * cfg.win
            lo, hi = starts[g0], starts[g0 + cfg.win]
            es = src_s[lo:hi]
            ed = dst_s[lo:hi]
            k = hi - lo
            ntile = int(Tw[w])
            cap = ntile * 128
            assert k <= cap
            # slot e -> partition e%128, tile e//128
            srcf = np.zeros(cap, np.int16)  # pad: row 0 (valid data)
            srcf[:k] = es.astype(np.int16)
            dstn = np.zeros(cap, np.int16)
            dstn[:k] = (ed - c * cfg.npc).astype(np.int16)
            dstw = np.full(cap, cfg.win - 1, np.int64)  # pad: last local node
            dstw[:k] = ed - g0
            ee = np.arange(cap)
            biasc[ee[:k] % 128, toff + ee[:k] // 128] = 0.0
            ohc[ee % 128, toff + ee // 128, dstw] = 1.0
            ohTc[dstw, (toff + ee // 128) * 128 + ee % 128] = 1.0
            # packed int16 gather indices: slot e at [e%16, (toff + e//128)*8 + (e%128)//16]
            col = (toff + ee // 128) * 8 + (ee % 128) // 16
            isc[ee % 16, col] = srcf
            idc[ee % 16, col] = dstn
            toff += ntile
        biass_l.append(biasc)
        ohs_l.append(ohc.reshape(128, T * cfg.win))
        ohsT_l.append(ohTc)
        # each of the 8 Q7 cores reads its own 16-partition stripe
        isrc_l.append(np.tile(isc[0:16], (8, 1)))
        idst_l.append(np.tile(idc[0:16], (8, 1)))

    def chunkmajor_T(a):  # [n, K] f32 -> [128, K//128 * n] f16 (p,k,n)
        n, K = a.shape
        kc = K // 128
        t = a.T.reshape(kc, 128, n).transpose(1, 0, 2).reshape(128, kc * n)
        return np.ascontiguousarray(t).astype(F16NP)

    def chunkfirst(a):  # [K, M] f32 -> [128, K//128 * M] f16
        Kd, M = a.shape
        kc = Kd // 128
        t = a.reshape(kc, 128, M).transpose(1, 0, 2).reshape(128, kc * M)
        return np.ascontiguousarray(t).astype(F16NP)

    x = np.asarray(x, np.float32)
    W1_l = np.asarray(W1_l, np.float32)[:, perm]
    W1_r = np.asarray(W1_r, np.float32)[:, perm]
    att1_p = np.asarray(att1, np.float32).reshape(-1)[perm]
    W2_l = np.asarray(W2_l, np.float32)[perm, :]
    W2_r = np.asarray(W2_r, np.float32)[perm, :]

    xt_np = chunkmajor_T(x)
    w1l_np = chunkfirst(W1_l)
    w1r_np = chunkfirst(W1_r)
    # combined [W2_l | W2_r] per 128-chunk
    w2l_c = W2_l.reshape(cfg.hcc, 128, cfg.out)
    w2r_c = W2_r.reshape(cfg.hcc, 128, cfg.out)
    w2lr = np.concatenate([w2l_c, w2r_c], axis=2)  # [hcc, 128, 64]
    w2lr_np = np.ascontiguousarray(
        w2lr.transpose(1, 0, 2).reshape(128, cfg.hcc * 64)
    ).astype(F16NP)

    att1_np = np.tile(att1_p[None, :], (128, 1)).astype(F16NP)
    att2_np = np.tile(
        np.asarray(att2, np.float32).reshape(1, cfg.out), (128, 1)
    ).astype(F16NP)
    clrr_np = np.tile(
        np.concatenate([W2_l.sum(0), W2_r.sum(0)])[None, :], (128, 1)
    ).astype(np.float32)

    b1 = np.asarray(b1, np.float32)
    b2 = np.asarray(b2, np.float32)
    add_b1 = bool(np.any(b1 != 0))
    add_b2 = bool(np.any(b2 != 0))
    b1r_np = np.tile(b1[perm][None, :], (128, 1)).astype(F16NP)
    b2r_np = np.tile(b2[None, :], (128, 1)).astype(np.float32)

    in_maps = []
    for c in range(cfg.ncores):
        lo = c * cfg.npc
        m = {
            "xt": xt_np,
            "xlt": chunkmajor_T(x[lo : lo + cfg.npc]),
            "w1l": w1l_np,
            "w1r": w1r_np,
            "w2lr": w2lr_np,
            "att1r": att1_np,
            "att2r": att2_np,
            "clrr": clrr_np,
            "biass": biass_l[c],
            "ohs": ohs_l[c],
            "ohsT": ohsT_l[c],
            "idx_src": isrc_l[c],
            "idx_dst": idst_l[c],
        }
        if add_b1:
            m["b1r"] = b1r_np
        if add_b2:
            m["b2r"] = b2r_np
        in_maps.append(m)
    return Tw, in_maps, add_b1, add_b2


_CACHE = {}


def _get_cfg():
    return Cfg(n=10000, in_ch=256, hid=128, heads=8, out_ch=32, ncores=8, win=125, nwin=10)


def kernel(x, edge_index, W1_l, W1_r, att1, b1, W2_l, W2_r, att2, b2, _trace=False):
    cfg = _get_cfg()
    Tw, in_maps, add_b1, add_b2 = host_prep(
        cfg, x, edge_index, W1_l, W1_r, att1, b1, W2_l, W2_r, att2, b2
    )
    key = (tuple(Tw), add_b1, add_b2)
    if key not in _CACHE:
        _CACHE[key] = build_program(cfg, Tw, add_b1, add_b2)
    nc = _CACHE[key]
    res = run_bass_kernel_spmd(
        nc, in_maps, list(range(cfg.ncores)), trace=bool(_trace)
    )
    if _trace:
        kernel.last_exec_time_ns = res.exec_time_ns
        kernel.last_results = res
    out = np.concatenate([res.results[c]["out"] for c in range(cfg.ncores)], axis=0)
    return out.astype(np.float32)


if __name__ == "__main__":
    cfg = _get_cfg()
    nc = build_program(cfg, [17] * 10, False, False)
    print("build ok")
